# revision 3
# baseline (speedup 1.0000x reference)
"""Trainium2 Bass kernel for NeuralFractionalDE.

out = x_current + drift(x)*DT + softplus_head(x)*(noise*DT^H) + frac_deriv*(ALPHA*DT)

where frac_deriv = sum_k (x_hist[:,k+1,:]-x_hist[:,k,:]) * w[k] collapses to
sum_t c[t] * x_hist[:,t,:] with c[t] = w[t-1]-w[t] (boundary adjusted).

Data parallel over 8 NeuronCores (256 batch rows each). The 128 MiB/core
x_history stream rides the HWDGE sync ring (RTL descriptor generation: no
Q7 emission serialization and no SWDGE descriptor-ring port contention,
which paces SDMA engine 15 ~13% slow and backpressures the other 15).
Time is laid out as t = 8*p + ti (p = partition) so each partition streams
contiguous 4 KiB rows from HBM; 8 accumulating [128,1]^T x [128,512] fp32
matmuls per psum row perform the weighted time reduction (2-pass fp32 PE
streaming ~273 us still clears the ~375 us HBM floor).

All weights/constants arrive as ONE packed [128, 2452] tensor via a single
SWDGE DMA; per-group frac results scatter SBUF->SBUF on the otherwise-idle
gpsimd ring (no DRAM scratch roundtrip).
"""

import math

import numpy as np

try:
    import concourse.bass as bass
except ImportError:  # pragma: no cover
    import sys

    sys.path.insert(0, "/opt/trn_rl_repo")
    import concourse.bass as bass

import concourse.bacc as bacc
import concourse.mybir as mybir
import concourse.tile as tile
from concourse.bass_utils import run_bass_kernel_spmd

ALPHA = 0.7
K = 1024
DT = 0.01
H = 0.5 + ALPHA / 2
D = 128
HID = 256
B = 2048
N_CORES = 8
B_PER = B // N_CORES  # 256
TI = 8  # time sub-steps per partition: t = TI*p + ti
NB = 4  # batch rows per streamed x_history tile
G = B_PER // NB  # 64 groups; batch b = NB*g + bi
GT = G // 2  # groups per 128-row output tile

F32 = mybir.dt.float32
AF = mybir.ActivationFunctionType
OP = mybir.AluOpType

# ---- packed-constant column layout ----
_C_DW1 = 0
_C_DW2 = 256
_C_DW3 = 768
_C_GW1 = 1024
_C_GW2 = 1280
_C_GW3 = 1792
_C_BIAS = 2048  # 10 cols: 2*db1(2) 2*db2(2) db3 2*gb1(2) 2*gb2(2) gb3
_C_C8 = 2058
_C_IDENT = 2066
_C_XC = 2194
_C_NZ = 2450
_C_TOT = 2452


def _coeffs() -> np.ndarray:
    t = np.arange(1, K + 1, dtype=np.float32)
    kern = (t ** np.float32(-ALPHA)) / np.float32(math.gamma(1.0 - ALPHA))
    w = kern[::-1][: K - 1]  # w[k] = kern[K-1-k]
    c = np.zeros(K, dtype=np.float32)
    c[1:] += w
    c[: K - 1] -= w
    c *= np.float32(ALPHA * DT)
    return np.ascontiguousarray(c.reshape(128, TI))  # c8[p, ti] = c[TI*p + ti]


def _build_program() -> bass.Bass:
    # Bacc (not raw Bass): its compile() legalizes semaphore waits to the
    # 1-wait-per-instruction ISA limit (generate_event_semaphores).
    nc = bacc.Bacc(None, target_bir_lowering=False)

    xh = nc.dram_tensor("xh", [B_PER, K, D], F32, kind="ExternalInput")
    pk = nc.dram_tensor("pk", [128, _C_TOT], F32, kind="ExternalInput")
    out = nc.dram_tensor("out", [B_PER, D], F32, kind="ExternalOutput")

    with tile.TileContext(nc) as tc:
        with (
            tc.tile_pool(name="const", bufs=1) as cpool,
            tc.tile_pool(name="stream", bufs=8) as spool,
            tc.tile_pool(name="work", bufs=4) as wpool,
            tc.tile_pool(name="psf", bufs=4, space=bass.MemorySpace.PSUM) as psf,
            tc.tile_pool(name="psm", bufs=2, space=bass.MemorySpace.PSUM) as psm,
            tc.tile_pool(name="pst", bufs=2, space=bass.MemorySpace.PSUM) as pst,
        ):
            # ---- one packed const load on the gpsimd (SWDGE) ring, keeping
            # both HWDGE rings clear for the stream ----
            pk_sb = cpool.tile([128, _C_TOT], F32, tag="pk")
            nc.gpsimd.dma_start(out=pk_sb[:], in_=pk[:])

            c8 = pk_sb[:, _C_C8 : _C_C8 + TI]
            ident = pk_sb[:, _C_IDENT : _C_IDENT + 128]
            xc = [pk_sb[:, _C_XC + tb * 128 : _C_XC + (tb + 1) * 128] for tb in range(2)]
            nz = [pk_sb[:, _C_NZ + tb : _C_NZ + tb + 1] for tb in range(2)]

            def wslices(base1, base2, base3, boff):
                w1 = pk_sb[:, base1 : base1 + HID]
                w2 = [pk_sb[:, base2 + i * HID : base2 + (i + 1) * HID] for i in range(2)]
                w3 = [pk_sb[:, base3 + i * D : base3 + (i + 1) * D] for i in range(2)]
                b1 = [pk_sb[:, _C_BIAS + boff + i : _C_BIAS + boff + i + 1] for i in range(2)]
                b2 = [pk_sb[:, _C_BIAS + boff + 2 + i : _C_BIAS + boff + 3 + i] for i in range(2)]
                b3 = pk_sb[:, _C_BIAS + boff + 4 : _C_BIAS + boff + 5]
                return w1, b1, w2, b2, w3, b3

            wsb = {
                "d": wslices(_C_DW1, _C_DW2, _C_DW3, 0),
                "g": wslices(_C_GW1, _C_GW2, _C_GW3, 5),
            }

            # ---- x_current transpose: [b, d] -> [d, b] ----
            xcT_sb = cpool.tile([128, B_PER], F32, tag="xcT")
            for tb in range(2):
                pt = pst.tile([128, 128], F32, tag="pst")
                nc.tensor.transpose(pt[:], xc[tb], ident)
                nc.scalar.activation(
                    xcT_sb[:, tb * 128 : (tb + 1) * 128], pt[:], AF.Copy
                )

            # ---- the two MLPs in feature-major layout ----
            # The compiler's ACT LUT sets have no {tanh, ln} combination and
            # no softplus at all, so everything uses natural_log_exp_and_others
            # ({exp, ln, copy}): tanh(y+b) = 1 - 2/(1 + exp(2y + 2b)) and
            # softplus(x+b) = ln(1 + exp(x + b)). Hidden biases arrive
            # pre-doubled in the packed tensor (bias of Exp must be 2*b).
            def tanh_act(out_ap, ps_ap, bias2_ap):
                nc.scalar.activation(out_ap, ps_ap, AF.Exp, bias=bias2_ap, scale=2.0)
                nc.vector.tensor_scalar(
                    out=out_ap, in0=out_ap, scalar1=1.0, scalar2=None, op0=OP.add
                )
                nc.vector.reciprocal(out_ap, out_ap)
                nc.vector.tensor_scalar(
                    out=out_ap,
                    in0=out_ap,
                    scalar1=-2.0,
                    scalar2=1.0,
                    op0=OP.mult,
                    op1=OP.add,
                )

            def mlp(net: str):
                w1, b1, w2, b2, w3, b3 = wsb[net]
                h1 = []
                for j in range(2):
                    ps = psm.tile([128, B_PER], F32, tag="psm")
                    nc.tensor.matmul(
                        ps[:],
                        w1[:, j * 128 : (j + 1) * 128],
                        xcT_sb[:],
                        start=True,
                        stop=True,
                    )
                    h = cpool.tile([128, B_PER], F32, tag=f"{net}h1{j}")
                    tanh_act(h[:], ps[:], b1[j])
                    h1.append(h)
                h2 = []
                for j in range(2):
                    ps = psm.tile([128, B_PER], F32, tag="psm")
                    for i in range(2):
                        nc.tensor.matmul(
                            ps[:],
                            w2[i][:, j * 128 : (j + 1) * 128],
                            h1[i][:],
                            start=(i == 0),
                            stop=(i == 1),
                        )
                    h = cpool.tile([128, B_PER], F32, tag=f"{net}h2{j}")
                    tanh_act(h[:], ps[:], b2[j])
                    h2.append(h)
                ps = psm.tile([128, B_PER], F32, tag="psm")
                for i in range(2):
                    nc.tensor.matmul(
                        ps[:], w3[i][:], h2[i][:], start=(i == 0), stop=(i == 1)
                    )
                return ps, b3

            driftT_sb = cpool.tile([128, B_PER], F32, tag="driftT")
            ps3, db3_sb = mlp("d")
            # driftT = (raw + b3) * DT
            nc.vector.tensor_scalar(
                out=driftT_sb[:],
                in0=ps3[:],
                scalar1=db3_sb,
                scalar2=float(DT),
                op0=OP.add,
                op1=OP.mult,
            )
            diffT_sb = cpool.tile([128, B_PER], F32, tag="diffT")
            ps3g, gb3_sb = mlp("g")
            # softplus via ln(1 + exp(x + b))
            nc.scalar.activation(diffT_sb[:], ps3g[:], AF.Exp, bias=gb3_sb)
            nc.vector.tensor_scalar(
                out=diffT_sb[:],
                in0=diffT_sb[:],
                scalar1=1.0,
                scalar2=None,
                op0=OP.add,
            )
            nc.scalar.activation(diffT_sb[:], diffT_sb[:], AF.Ln)

            # o_base[tb] = xc + drift*DT + diffusion*noise*DT^H, computed as
            # soon as the MLPs finish; the frac term lands later via scatters.
            o_base = []
            o_frac = []
            for tb in range(2):
                ptd = pst.tile([128, 128], F32, tag="pst")
                nc.tensor.transpose(
                    ptd[:], driftT_sb[:, tb * 128 : (tb + 1) * 128], ident
                )
                ptg = pst.tile([128, 128], F32, tag="pst")
                nc.tensor.transpose(
                    ptg[:], diffT_sb[:, tb * 128 : (tb + 1) * 128], ident
                )
                ob = cpool.tile([128, D], F32, tag=f"obase{tb}")
                nc.vector.tensor_scalar(
                    out=ob[:],
                    in0=ptg[:],
                    scalar1=nz[tb],
                    scalar2=float(DT**H),
                    op0=OP.mult,
                    op1=OP.mult,
                )
                nc.vector.tensor_add(out=ob[:], in0=ob[:], in1=ptd[:])
                nc.vector.tensor_add(out=ob[:], in0=ob[:], in1=xc[tb])
                o_base.append(ob)
                of = cpool.tile([128, D], F32, tag=f"ofrac{tb}")
                o_frac.append(of)

            # ---- fractional-derivative stream: the 128 MiB x_history scan ----
            # xh[b, TI*p + ti, d] -> tile[p, bi, ti, d] for b = NB*g + bi, so
            # each partition reads NB contiguous 4 KiB rows per tile.
            xh_r = xh.rearrange("(g bi) (p ti) d -> g p bi ti d", bi=NB, p=128)
            for g in range(G):
                tb, gi = divmod(g, GT)
                xt = spool.tile([128, NB, TI, D], F32, tag="xt")
                nc.sync.dma_start(out=xt[:], in_=xh_r[g])
                ps = psf.tile([1, NB * D], F32, tag="psf")
                for ti in range(TI):
                    nc.tensor.matmul(
                        ps[:],
                        c8[:, ti : ti + 1],
                        xt[:, :, ti, :],
                        start=(ti == 0),
                        stop=(ti == TI - 1),
                    )
                stage = wpool.tile([1, NB * D], F32, tag="stage")
                nc.scalar.activation(stage[:], ps[:], AF.Copy)
                # scatter rows b = NB*g + bi into the output-tile accumulator
                # (SBUF->SBUF on the idle SWDGE ring)
                nc.gpsimd.dma_start(
                    out=o_frac[tb][NB * gi : NB * (gi + 1), :],
                    in_=stage[0:1].rearrange("o (bi d) -> o bi d", bi=NB),
                )
                if gi == GT - 1:
                    o = wpool.tile([128, D], F32, tag="o")
                    nc.vector.tensor_add(
                        out=o[:], in0=o_base[tb][:], in1=o_frac[tb][:]
                    )
                    nc.gpsimd.dma_start(
                        out=out[tb * 128 : (tb + 1) * 128, :], in_=o[:]
                    )

    nc.compile()
    return nc


_NC_CACHE = None


def _get_program() -> bass.Bass:
    global _NC_CACHE
    if _NC_CACHE is None:
        _NC_CACHE = _build_program()
    return _NC_CACHE


def _pack_consts(inputs: dict) -> np.ndarray:
    f = lambda x: np.asarray(x, dtype=np.float32)
    pk = np.zeros((128, _C_TOT), dtype=np.float32)
    pk[:, _C_DW1 : _C_DW1 + HID] = f(inputs["dw1"])
    pk[:, _C_DW2 : _C_DW2 + 2 * HID] = f(inputs["dw2"]).reshape(2, 128, HID).transpose(1, 0, 2).reshape(128, 2 * HID)
    pk[:, _C_DW3 : _C_DW3 + 2 * D] = f(inputs["dw3"]).reshape(2, 128, D).transpose(1, 0, 2).reshape(128, 2 * D)
    pk[:, _C_GW1 : _C_GW1 + HID] = f(inputs["gw1"])
    pk[:, _C_GW2 : _C_GW2 + 2 * HID] = f(inputs["gw2"]).reshape(2, 128, HID).transpose(1, 0, 2).reshape(128, 2 * HID)
    pk[:, _C_GW3 : _C_GW3 + 2 * D] = f(inputs["gw3"]).reshape(2, 128, D).transpose(1, 0, 2).reshape(128, 2 * D)
    bias = np.stack(
        [
            2.0 * f(inputs["db1"])[:128],
            2.0 * f(inputs["db1"])[128:],
            2.0 * f(inputs["db2"])[:128],
            2.0 * f(inputs["db2"])[128:],
            f(inputs["db3"]),
            2.0 * f(inputs["gb1"])[:128],
            2.0 * f(inputs["gb1"])[128:],
            2.0 * f(inputs["gb2"])[:128],
            2.0 * f(inputs["gb2"])[128:],
            f(inputs["gb3"]),
        ],
        axis=1,
    )
    pk[:, _C_BIAS : _C_BIAS + 10] = bias
    pk[:, _C_C8 : _C_C8 + TI] = _coeffs()
    pk[:, _C_IDENT : _C_IDENT + 128] = np.eye(128, dtype=np.float32)
    return pk


def _in_maps(inputs: dict) -> list[dict]:
    f = lambda x: np.ascontiguousarray(np.asarray(x, dtype=np.float32))
    xh = f(inputs["x_history"])
    xc = f(inputs["x_current"])
    nz = f(inputs["noise"])
    assert xh.shape == (B, K, D) and xc.shape == (B, D) and nz.shape == (B,)
    pk_base = _pack_consts(inputs)
    maps = []
    for c in range(N_CORES):
        s = slice(c * B_PER, (c + 1) * B_PER)
        pk = pk_base.copy()
        # xc tiles are stored [p, d] row-major per 128-row block
        pk[:, _C_XC : _C_XC + 128] = xc[s][:128]
        pk[:, _C_XC + 128 : _C_XC + 256] = xc[s][128:]
        pk[:, _C_NZ] = nz[s][:128]
        pk[:, _C_NZ + 1] = nz[s][128:]
        maps.append({"xh": xh[s], "pk": np.ascontiguousarray(pk)})
    return maps


def run(inputs: dict, trace: bool = False):
    nc = _get_program()
    res = run_bass_kernel_spmd(nc, _in_maps(inputs), list(range(N_CORES)), trace=trace)
    out = np.concatenate([res.results[c]["out"] for c in range(N_CORES)], axis=0)
    return out, res


def kernel(**inputs) -> np.ndarray:
    out, _ = run(inputs, trace=False)
    return out


# revision 7
# speedup vs baseline: 1.1272x; 1.1272x over previous
"""Trainium2 Bass kernel for NeuralFractionalDE.

out = x_current + drift(x)*DT + softplus_head(x)*(noise*DT^H) + frac_deriv*(ALPHA*DT)

where frac_deriv = sum_k (x_hist[:,k+1,:]-x_hist[:,k,:]) * w[k] collapses to
sum_t c[t] * x_hist[:,t,:] with c[t] = w[t-1]-w[t] (boundary adjusted).

Data parallel over 8 NeuronCores (256 batch rows each). The 128 MiB/core
x_history stream rides the HWDGE sync ring (RTL descriptor generation: no
Q7 emission serialization and no SWDGE descriptor-ring port contention,
which paces SDMA engine 15 ~13% slow and backpressures the other 15).
Time is laid out as t = 8*p + ti (p = partition) so each partition streams
contiguous 4 KiB rows from HBM; 8 accumulating [128,1]^T x [128,512] fp32
matmuls per psum row perform the weighted time reduction (2-pass fp32 PE
streaming ~273 us still clears the ~375 us HBM floor).

All weights/constants arrive as ONE packed [128, 2452] tensor via a single
SWDGE DMA; per-group frac results scatter SBUF->SBUF on the otherwise-idle
gpsimd ring (no DRAM scratch roundtrip).
"""

import math

import numpy as np

try:
    import concourse.bass as bass
except ImportError:  # pragma: no cover
    import sys

    sys.path.insert(0, "/opt/trn_rl_repo")
    import concourse.bass as bass

import concourse.bacc as bacc
import concourse.mybir as mybir
import concourse.tile as tile
from concourse.bass_utils import run_bass_kernel_spmd

ALPHA = 0.7
K = 1024
DT = 0.01
H = 0.5 + ALPHA / 2
D = 128
HID = 256
B = 2048
N_CORES = 8
B_PER = B // N_CORES  # 256
TI = 8  # time sub-steps per partition: t = TI*p + ti
NB = 4  # batch rows per streamed x_history tile
G = B_PER // NB  # 64 groups; batch b = NB*g + bi
GT = G // 2  # groups per 128-row output tile

F32 = mybir.dt.float32
BF16 = mybir.dt.bfloat16
AF = mybir.ActivationFunctionType
OP = mybir.AluOpType

# ---- packed-constant column layout ----
_C_DW1 = 0
_C_DW2 = 256
_C_DW3 = 768
_C_GW1 = 1024
_C_GW2 = 1280
_C_GW3 = 1792
_C_BIAS = 2048  # 10 cols: 2*db1(2) 2*db2(2) db3 2*gb1(2) 2*gb2(2) gb3
_C_C8 = 2058
_C_IDENT = 2066
_C_XC = 2194
_C_NZ = 2450
_C_TOT = 2452


def _coeffs() -> np.ndarray:
    t = np.arange(1, K + 1, dtype=np.float32)
    kern = (t ** np.float32(-ALPHA)) / np.float32(math.gamma(1.0 - ALPHA))
    w = kern[::-1][: K - 1]  # w[k] = kern[K-1-k]
    c = np.zeros(K, dtype=np.float32)
    c[1:] += w
    c[: K - 1] -= w
    c *= np.float32(ALPHA * DT)
    return np.ascontiguousarray(c.reshape(128, TI))  # c8[p, ti] = c[TI*p + ti]


def _build_program() -> bass.Bass:
    # Bacc (not raw Bass): its compile() legalizes semaphore waits to the
    # 1-wait-per-instruction ISA limit (generate_event_semaphores).
    nc = bacc.Bacc(None, target_bir_lowering=False)

    xh = nc.dram_tensor("xh", [B_PER, K, D], F32, kind="ExternalInput")
    pk = nc.dram_tensor("pk", [128, _C_TOT], F32, kind="ExternalInput")
    out = nc.dram_tensor("out", [B_PER, D], F32, kind="ExternalOutput")

    with tile.TileContext(nc) as tc:
        with (
            tc.tile_pool(name="const", bufs=1) as cpool,
            tc.tile_pool(name="stream", bufs=6) as spool,
            tc.tile_pool(name="stream16", bufs=4) as spool16,
            tc.tile_pool(name="work", bufs=4) as wpool,
            tc.tile_pool(name="psf", bufs=4, space=bass.MemorySpace.PSUM) as psf,
            tc.tile_pool(name="psm", bufs=2, space=bass.MemorySpace.PSUM) as psm,
            tc.tile_pool(name="pst", bufs=2, space=bass.MemorySpace.PSUM) as pst,
        ):
            # ---- one packed const load on the gpsimd (SWDGE) ring, keeping
            # both HWDGE rings clear for the stream ----
            pk_sb = cpool.tile([128, _C_TOT], F32, tag="pk")
            nc.gpsimd.dma_start(out=pk_sb[:], in_=pk[:])

            # bf16 stationary for the stream matmuls (moving operand is bf16:
            # fp32 moving runs a 2-pass ~1.4 GHz decomposition, 3.4x slower)
            c8b = cpool.tile([128, TI], BF16, tag="c8b")
            nc.scalar.copy(c8b[:], pk_sb[:, _C_C8 : _C_C8 + TI])
            ident = pk_sb[:, _C_IDENT : _C_IDENT + 128]
            xc = [pk_sb[:, _C_XC + tb * 128 : _C_XC + (tb + 1) * 128] for tb in range(2)]
            nz = [pk_sb[:, _C_NZ + tb : _C_NZ + tb + 1] for tb in range(2)]

            def wslices(base1, base2, base3, boff):
                w1 = pk_sb[:, base1 : base1 + HID]
                w2 = [pk_sb[:, base2 + i * HID : base2 + (i + 1) * HID] for i in range(2)]
                w3 = [pk_sb[:, base3 + i * D : base3 + (i + 1) * D] for i in range(2)]
                b1 = [pk_sb[:, _C_BIAS + boff + i : _C_BIAS + boff + i + 1] for i in range(2)]
                b2 = [pk_sb[:, _C_BIAS + boff + 2 + i : _C_BIAS + boff + 3 + i] for i in range(2)]
                b3 = pk_sb[:, _C_BIAS + boff + 4 : _C_BIAS + boff + 5]
                return w1, b1, w2, b2, w3, b3

            wsb = {
                "d": wslices(_C_DW1, _C_DW2, _C_DW3, 0),
                "g": wslices(_C_GW1, _C_GW2, _C_GW3, 5),
            }

            # ---- x_current transpose: [b, d] -> [d, b] ----
            xcT_sb = cpool.tile([128, B_PER], F32, tag="xcT")
            for tb in range(2):
                pt = pst.tile([128, 128], F32, tag="pst")
                nc.tensor.transpose(pt[:], xc[tb], ident)
                nc.scalar.activation(
                    xcT_sb[:, tb * 128 : (tb + 1) * 128], pt[:], AF.Copy
                )

            # ---- the two MLPs in feature-major layout ----
            # The compiler's ACT LUT sets have no {tanh, ln} combination and
            # no softplus at all, so everything uses natural_log_exp_and_others
            # ({exp, ln, copy}): tanh(y+b) = 1 - 2/(1 + exp(2y + 2b)) and
            # softplus(x+b) = ln(1 + exp(x + b)). Hidden biases arrive
            # pre-doubled in the packed tensor (bias of Exp must be 2*b).
            def tanh_act(out_ap, ps_ap, bias2_ap):
                nc.scalar.activation(out_ap, ps_ap, AF.Exp, bias=bias2_ap, scale=2.0)
                nc.vector.tensor_scalar(
                    out=out_ap, in0=out_ap, scalar1=1.0, scalar2=None, op0=OP.add
                )
                nc.vector.reciprocal(out_ap, out_ap)
                nc.vector.tensor_scalar(
                    out=out_ap,
                    in0=out_ap,
                    scalar1=-2.0,
                    scalar2=1.0,
                    op0=OP.mult,
                    op1=OP.add,
                )

            def mlp(net: str):
                w1, b1, w2, b2, w3, b3 = wsb[net]
                h1 = []
                for j in range(2):
                    ps = psm.tile([128, B_PER], F32, tag="psm")
                    nc.tensor.matmul(
                        ps[:],
                        w1[:, j * 128 : (j + 1) * 128],
                        xcT_sb[:],
                        start=True,
                        stop=True,
                    )
                    h = cpool.tile([128, B_PER], F32, tag=f"{net}h1{j}")
                    tanh_act(h[:], ps[:], b1[j])
                    h1.append(h)
                h2 = []
                for j in range(2):
                    ps = psm.tile([128, B_PER], F32, tag="psm")
                    for i in range(2):
                        nc.tensor.matmul(
                            ps[:],
                            w2[i][:, j * 128 : (j + 1) * 128],
                            h1[i][:],
                            start=(i == 0),
                            stop=(i == 1),
                        )
                    h = cpool.tile([128, B_PER], F32, tag=f"{net}h2{j}")
                    tanh_act(h[:], ps[:], b2[j])
                    h2.append(h)
                ps = psm.tile([128, B_PER], F32, tag="psm")
                for i in range(2):
                    nc.tensor.matmul(
                        ps[:], w3[i][:], h2[i][:], start=(i == 0), stop=(i == 1)
                    )
                return ps, b3

            driftT_sb = cpool.tile([128, B_PER], F32, tag="driftT")
            ps3, db3_sb = mlp("d")
            # driftT = (raw + b3) * DT
            nc.vector.tensor_scalar(
                out=driftT_sb[:],
                in0=ps3[:],
                scalar1=db3_sb,
                scalar2=float(DT),
                op0=OP.add,
                op1=OP.mult,
            )
            diffT_sb = cpool.tile([128, B_PER], F32, tag="diffT")
            ps3g, gb3_sb = mlp("g")
            # softplus via ln(1 + exp(x + b))
            nc.scalar.activation(diffT_sb[:], ps3g[:], AF.Exp, bias=gb3_sb)
            nc.vector.tensor_scalar(
                out=diffT_sb[:],
                in0=diffT_sb[:],
                scalar1=1.0,
                scalar2=None,
                op0=OP.add,
            )
            nc.scalar.activation(diffT_sb[:], diffT_sb[:], AF.Ln)

            # o_base[tb] = xc + drift*DT + diffusion*noise*DT^H, computed as
            # soon as the MLPs finish; the frac term lands later via scatters.
            o_base = []
            o_frac = []
            for tb in range(2):
                ptd = pst.tile([128, 128], F32, tag="pst")
                nc.tensor.transpose(
                    ptd[:], driftT_sb[:, tb * 128 : (tb + 1) * 128], ident
                )
                ptg = pst.tile([128, 128], F32, tag="pst")
                nc.tensor.transpose(
                    ptg[:], diffT_sb[:, tb * 128 : (tb + 1) * 128], ident
                )
                ob = cpool.tile([128, D], F32, tag=f"obase{tb}")
                nc.vector.tensor_scalar(
                    out=ob[:],
                    in0=ptg[:],
                    scalar1=nz[tb],
                    scalar2=float(DT**H),
                    op0=OP.mult,
                    op1=OP.mult,
                )
                nc.vector.tensor_add(out=ob[:], in0=ob[:], in1=ptd[:])
                nc.vector.tensor_add(out=ob[:], in0=ob[:], in1=xc[tb])
                o_base.append(ob)
                of = cpool.tile([128, D], F32, tag=f"ofrac{tb}")
                o_frac.append(of)

            # ---- fractional-derivative stream: the 128 MiB x_history scan ----
            # xh[b, TI*p + ti, d] -> tile[p, bi, ti, d] for b = NB*g + bi, so
            # each partition reads NB contiguous 4 KiB rows per tile.
            xh_r = xh.rearrange("(g bi) (p ti) d -> g p bi ti d", bi=NB, p=128)
            for g in range(G):
                tb, gi = divmod(g, GT)
                xt = spool.tile([128, NB, TI, D], F32, tag="xt")
                nc.sync.dma_start(out=xt[:], in_=xh_r[g])
                # fp32 -> bf16 on-chip (HWDGE cannot cast in flight); alternate
                # DVE / ACT so neither becomes the pacer
                xt16 = spool16.tile([128, NB, TI, D], BF16, tag="xt16")
                if g % 2 == 0:
                    nc.vector.tensor_scalar_mul(xt16[:], xt[:], 1.0)
                else:
                    nc.scalar.copy(xt16[:], xt[:])
                ps = psf.tile([1, NB * D], F32, tag="psf")
                for ti in range(TI):
                    nc.tensor.matmul(
                        ps[:],
                        c8b[:, ti : ti + 1],
                        xt16[:, :, ti, :],
                        start=(ti == 0),
                        stop=(ti == TI - 1),
                    )
                stage = wpool.tile([1, NB * D], F32, tag="stage")
                nc.scalar.activation(stage[:], ps[:], AF.Copy)
                # scatter rows b = NB*g + bi into the output-tile accumulator
                # (SBUF->SBUF on the idle SWDGE ring)
                nc.gpsimd.dma_start(
                    out=o_frac[tb][NB * gi : NB * (gi + 1), :],
                    in_=stage[0:1].rearrange("o (bi d) -> o bi d", bi=NB),
                )
                if gi == GT - 1:
                    o = wpool.tile([128, D], F32, tag="o")
                    nc.vector.tensor_add(
                        out=o[:], in0=o_base[tb][:], in1=o_frac[tb][:]
                    )
                    nc.gpsimd.dma_start(
                        out=out[tb * 128 : (tb + 1) * 128, :], in_=o[:]
                    )

    nc.compile()
    return nc


_NC_CACHE = None


def _get_program() -> bass.Bass:
    global _NC_CACHE
    if _NC_CACHE is None:
        _NC_CACHE = _build_program()
    return _NC_CACHE


def _pack_consts(inputs: dict) -> np.ndarray:
    f = lambda x: np.asarray(x, dtype=np.float32)
    pk = np.zeros((128, _C_TOT), dtype=np.float32)
    pk[:, _C_DW1 : _C_DW1 + HID] = f(inputs["dw1"])
    pk[:, _C_DW2 : _C_DW2 + 2 * HID] = f(inputs["dw2"]).reshape(2, 128, HID).transpose(1, 0, 2).reshape(128, 2 * HID)
    pk[:, _C_DW3 : _C_DW3 + 2 * D] = f(inputs["dw3"]).reshape(2, 128, D).transpose(1, 0, 2).reshape(128, 2 * D)
    pk[:, _C_GW1 : _C_GW1 + HID] = f(inputs["gw1"])
    pk[:, _C_GW2 : _C_GW2 + 2 * HID] = f(inputs["gw2"]).reshape(2, 128, HID).transpose(1, 0, 2).reshape(128, 2 * HID)
    pk[:, _C_GW3 : _C_GW3 + 2 * D] = f(inputs["gw3"]).reshape(2, 128, D).transpose(1, 0, 2).reshape(128, 2 * D)
    bias = np.stack(
        [
            2.0 * f(inputs["db1"])[:128],
            2.0 * f(inputs["db1"])[128:],
            2.0 * f(inputs["db2"])[:128],
            2.0 * f(inputs["db2"])[128:],
            f(inputs["db3"]),
            2.0 * f(inputs["gb1"])[:128],
            2.0 * f(inputs["gb1"])[128:],
            2.0 * f(inputs["gb2"])[:128],
            2.0 * f(inputs["gb2"])[128:],
            f(inputs["gb3"]),
        ],
        axis=1,
    )
    pk[:, _C_BIAS : _C_BIAS + 10] = bias
    pk[:, _C_C8 : _C_C8 + TI] = _coeffs()
    pk[:, _C_IDENT : _C_IDENT + 128] = np.eye(128, dtype=np.float32)
    return pk


def _in_maps(inputs: dict) -> list[dict]:
    f = lambda x: np.ascontiguousarray(np.asarray(x, dtype=np.float32))
    xh = f(inputs["x_history"])
    xc = f(inputs["x_current"])
    nz = f(inputs["noise"])
    assert xh.shape == (B, K, D) and xc.shape == (B, D) and nz.shape == (B,)
    pk_base = _pack_consts(inputs)
    maps = []
    for c in range(N_CORES):
        s = slice(c * B_PER, (c + 1) * B_PER)
        pk = pk_base.copy()
        # xc tiles are stored [p, d] row-major per 128-row block
        pk[:, _C_XC : _C_XC + 128] = xc[s][:128]
        pk[:, _C_XC + 128 : _C_XC + 256] = xc[s][128:]
        pk[:, _C_NZ] = nz[s][:128]
        pk[:, _C_NZ + 1] = nz[s][128:]
        maps.append({"xh": xh[s], "pk": np.ascontiguousarray(pk)})
    return maps


def run(inputs: dict, trace: bool = False):
    nc = _get_program()
    res = run_bass_kernel_spmd(nc, _in_maps(inputs), list(range(N_CORES)), trace=trace)
    out = np.concatenate([res.results[c]["out"] for c in range(N_CORES)], axis=0)
    return out, res


def kernel(**inputs) -> np.ndarray:
    out, _ = run(inputs, trace=False)
    return out


# revision 10
# speedup vs baseline: 1.1320x; 1.0042x over previous
"""Trainium2 Bass kernel for NeuralFractionalDE.

out = x_current + drift(x)*DT + softplus_head(x)*(noise*DT^H) + frac_deriv*(ALPHA*DT)

where frac_deriv = sum_k (x_hist[:,k+1,:]-x_hist[:,k,:]) * w[k] collapses to
sum_t c[t] * x_hist[:,t,:] with c[t] = w[t-1]-w[t] (boundary adjusted).

Data parallel over 8 NeuronCores (256 batch rows each). The 128 MiB/core
x_history stream rides the HWDGE sync ring (RTL descriptor generation: no
Q7 emission serialization and no SWDGE descriptor-ring port contention,
which paces SDMA engine 15 ~13% slow and backpressures the other 15).
Time is laid out as t = 8*p + ti (p = partition) so each partition streams
contiguous 4 KiB rows from HBM; 8 accumulating [128,1]^T x [128,512] fp32
matmuls per psum row perform the weighted time reduction (2-pass fp32 PE
streaming ~273 us still clears the ~375 us HBM floor).

All weights/constants arrive as ONE packed [128, 2452] tensor via a single
SWDGE DMA; per-group frac results scatter SBUF->SBUF on the otherwise-idle
gpsimd ring (no DRAM scratch roundtrip).
"""

import math

import numpy as np

try:
    import concourse.bass as bass
except ImportError:  # pragma: no cover
    import sys

    sys.path.insert(0, "/opt/trn_rl_repo")
    import concourse.bass as bass

import concourse.bacc as bacc
import concourse.mybir as mybir
import concourse.tile as tile
from concourse.bass_utils import run_bass_kernel_spmd

ALPHA = 0.7
K = 1024
DT = 0.01
H = 0.5 + ALPHA / 2
D = 128
HID = 256
B = 2048
N_CORES = 8
B_PER = B // N_CORES  # 256
TI = 8  # time sub-steps per partition: t = TI*p + ti
NB = 4  # batch rows per streamed x_history tile
G = B_PER // NB  # 64 groups; batch b = NB*g + bi
GT = G // 2  # groups per 128-row output tile

F32 = mybir.dt.float32
BF16 = mybir.dt.bfloat16
AF = mybir.ActivationFunctionType
OP = mybir.AluOpType

# ---- packed-constant column layout ----
_C_DW1 = 0
_C_DW2 = 256
_C_DW3 = 768
_C_GW1 = 1024
_C_GW2 = 1280
_C_GW3 = 1792
_C_BIAS = 2048  # 10 cols: 2*db1(2) 2*db2(2) db3 2*gb1(2) 2*gb2(2) gb3
_C_C8 = 2058
_C_IDENT = 2066
_C_XC = 2194
_C_NZ = 2450
_C_TOT = 2452


def _coeffs() -> np.ndarray:
    t = np.arange(1, K + 1, dtype=np.float32)
    kern = (t ** np.float32(-ALPHA)) / np.float32(math.gamma(1.0 - ALPHA))
    w = kern[::-1][: K - 1]  # w[k] = kern[K-1-k]
    c = np.zeros(K, dtype=np.float32)
    c[1:] += w
    c[: K - 1] -= w
    c *= np.float32(ALPHA * DT)
    return np.ascontiguousarray(c.reshape(128, TI))  # c8[p, ti] = c[TI*p + ti]


def _build_program() -> bass.Bass:
    # Bacc (not raw Bass): its compile() legalizes semaphore waits to the
    # 1-wait-per-instruction ISA limit (generate_event_semaphores).
    nc = bacc.Bacc(None, target_bir_lowering=False)

    xh = nc.dram_tensor("xh", [B_PER, K, D], F32, kind="ExternalInput")
    pk = nc.dram_tensor("pk", [128, _C_TOT], F32, kind="ExternalInput")
    out = nc.dram_tensor("out", [B_PER, D], F32, kind="ExternalOutput")

    with tile.TileContext(nc) as tc:
        with (
            tc.tile_pool(name="const", bufs=1) as cpool,
            tc.tile_pool(name="stream", bufs=6) as spool,
            tc.tile_pool(name="stream16", bufs=4) as spool16,
            tc.tile_pool(name="work", bufs=4) as wpool,
            tc.tile_pool(name="psf", bufs=4, space=bass.MemorySpace.PSUM) as psf,
            tc.tile_pool(name="psm", bufs=2, space=bass.MemorySpace.PSUM) as psm,
            tc.tile_pool(name="pst", bufs=2, space=bass.MemorySpace.PSUM) as pst,
        ):
            # ---- one packed const load on the gpsimd (SWDGE) ring, keeping
            # both HWDGE rings clear for the stream ----
            pk_sb = cpool.tile([128, _C_TOT], F32, tag="pk")
            nc.gpsimd.dma_start(out=pk_sb[:], in_=pk[:])

            # bf16 stationary for the stream matmuls (moving operand is bf16:
            # fp32 moving runs a 2-pass ~1.4 GHz decomposition, 3.4x slower)
            c8b = cpool.tile([128, TI], BF16, tag="c8b")
            nc.scalar.copy(c8b[:], pk_sb[:, _C_C8 : _C_C8 + TI])
            ident = pk_sb[:, _C_IDENT : _C_IDENT + 128]
            xc = [pk_sb[:, _C_XC + tb * 128 : _C_XC + (tb + 1) * 128] for tb in range(2)]
            nz = [pk_sb[:, _C_NZ + tb : _C_NZ + tb + 1] for tb in range(2)]

            def wslices(base1, base2, base3, boff):
                w1 = pk_sb[:, base1 : base1 + HID]
                w2 = [pk_sb[:, base2 + i * HID : base2 + (i + 1) * HID] for i in range(2)]
                w3 = [pk_sb[:, base3 + i * D : base3 + (i + 1) * D] for i in range(2)]
                b1 = [pk_sb[:, _C_BIAS + boff + i : _C_BIAS + boff + i + 1] for i in range(2)]
                b2 = [pk_sb[:, _C_BIAS + boff + 2 + i : _C_BIAS + boff + 3 + i] for i in range(2)]
                b3 = pk_sb[:, _C_BIAS + boff + 4 : _C_BIAS + boff + 5]
                return w1, b1, w2, b2, w3, b3

            wsb = {
                "d": wslices(_C_DW1, _C_DW2, _C_DW3, 0),
                "g": wslices(_C_GW1, _C_GW2, _C_GW3, 5),
            }

            # ---- x_current transpose: [b, d] -> [d, b] ----
            xcT_sb = cpool.tile([128, B_PER], F32, tag="xcT")
            for tb in range(2):
                pt = pst.tile([128, 128], F32, tag="pst")
                nc.tensor.transpose(pt[:], xc[tb], ident)
                nc.scalar.activation(
                    xcT_sb[:, tb * 128 : (tb + 1) * 128], pt[:], AF.Copy
                )

            # ---- the two MLPs in feature-major layout ----
            # The compiler's ACT LUT sets have no {tanh, ln} combination and
            # no softplus at all, so everything uses natural_log_exp_and_others
            # ({exp, ln, copy}): tanh(y+b) = 1 - 2/(1 + exp(2y + 2b)) and
            # softplus(x+b) = ln(1 + exp(x + b)). Hidden biases arrive
            # pre-doubled in the packed tensor (bias of Exp must be 2*b).
            def tanh_act(out_ap, ps_ap, bias2_ap):
                nc.scalar.activation(out_ap, ps_ap, AF.Exp, bias=bias2_ap, scale=2.0)
                nc.vector.tensor_scalar(
                    out=out_ap, in0=out_ap, scalar1=1.0, scalar2=None, op0=OP.add
                )
                nc.vector.reciprocal(out_ap, out_ap)
                nc.vector.tensor_scalar(
                    out=out_ap,
                    in0=out_ap,
                    scalar1=-2.0,
                    scalar2=1.0,
                    op0=OP.mult,
                    op1=OP.add,
                )

            def mlp(net: str):
                w1, b1, w2, b2, w3, b3 = wsb[net]
                h1 = []
                for j in range(2):
                    ps = psm.tile([128, B_PER], F32, tag="psm")
                    nc.tensor.matmul(
                        ps[:],
                        w1[:, j * 128 : (j + 1) * 128],
                        xcT_sb[:],
                        start=True,
                        stop=True,
                    )
                    h = cpool.tile([128, B_PER], F32, tag=f"{net}h1{j}")
                    tanh_act(h[:], ps[:], b1[j])
                    h1.append(h)
                h2 = []
                for j in range(2):
                    ps = psm.tile([128, B_PER], F32, tag="psm")
                    for i in range(2):
                        nc.tensor.matmul(
                            ps[:],
                            w2[i][:, j * 128 : (j + 1) * 128],
                            h1[i][:],
                            start=(i == 0),
                            stop=(i == 1),
                        )
                    h = cpool.tile([128, B_PER], F32, tag=f"{net}h2{j}")
                    tanh_act(h[:], ps[:], b2[j])
                    h2.append(h)
                ps = psm.tile([128, B_PER], F32, tag="psm")
                for i in range(2):
                    nc.tensor.matmul(
                        ps[:], w3[i][:], h2[i][:], start=(i == 0), stop=(i == 1)
                    )
                return ps, b3

            driftT_sb = cpool.tile([128, B_PER], F32, tag="driftT")
            ps3, db3_sb = mlp("d")
            # driftT = (raw + b3) * DT
            nc.vector.tensor_scalar(
                out=driftT_sb[:],
                in0=ps3[:],
                scalar1=db3_sb,
                scalar2=float(DT),
                op0=OP.add,
                op1=OP.mult,
            )
            diffT_sb = cpool.tile([128, B_PER], F32, tag="diffT")
            ps3g, gb3_sb = mlp("g")
            # softplus via ln(1 + exp(x + b))
            nc.scalar.activation(diffT_sb[:], ps3g[:], AF.Exp, bias=gb3_sb)
            nc.vector.tensor_scalar(
                out=diffT_sb[:],
                in0=diffT_sb[:],
                scalar1=1.0,
                scalar2=None,
                op0=OP.add,
            )
            nc.scalar.activation(diffT_sb[:], diffT_sb[:], AF.Ln)

            # o_base[tb] = xc + drift*DT + diffusion*noise*DT^H, computed as
            # soon as the MLPs finish; the frac term lands later via scatters.
            o_base = []
            o_frac = []
            for tb in range(2):
                ptd = pst.tile([128, 128], F32, tag="pst")
                nc.tensor.transpose(
                    ptd[:], driftT_sb[:, tb * 128 : (tb + 1) * 128], ident
                )
                ptg = pst.tile([128, 128], F32, tag="pst")
                nc.tensor.transpose(
                    ptg[:], diffT_sb[:, tb * 128 : (tb + 1) * 128], ident
                )
                ob = cpool.tile([128, D], F32, tag=f"obase{tb}")
                nc.vector.tensor_scalar(
                    out=ob[:],
                    in0=ptg[:],
                    scalar1=nz[tb],
                    scalar2=float(DT**H),
                    op0=OP.mult,
                    op1=OP.mult,
                )
                nc.vector.tensor_add(out=ob[:], in0=ob[:], in1=ptd[:])
                nc.vector.tensor_add(out=ob[:], in0=ob[:], in1=xc[tb])
                o_base.append(ob)
                of = cpool.tile([128, D], F32, tag=f"ofrac{tb}")
                o_frac.append(of)

            # ---- fractional-derivative stream: the 128 MiB x_history scan ----
            # xh[b, TI*p + ti, d] -> tile[p, bi, ti, d] for b = NB*g + bi, so
            # each partition reads NB contiguous 4 KiB rows per tile.
            # All casts ride ACT (dedicated SBUF ports): big DVE ops measurably
            # slow SDMA engine 15's packets (p90 158->280 ns), making it pace
            # the whole stream. Scatters/outs ride the sparse SWDGE ring.
            def do_group(dma_ap, nb, tb, row0):
                # full-size tiles with fixed tags (one ring per pool even when
                # the final nb=2 sub-groups only fill half)
                xt = spool.tile([128, NB, TI, D], F32, tag="xt")
                nc.sync.dma_start(out=xt[:, 0:nb], in_=dma_ap)
                xt16 = spool16.tile([128, NB, TI, D], BF16, tag="xt16")
                nc.scalar.copy(xt16[:, 0:nb], xt[:, 0:nb])
                ps = psf.tile([1, NB * D], F32, tag="psf")
                for ti in range(TI):
                    nc.tensor.matmul(
                        ps[0:1, 0 : nb * D],
                        c8b[:, ti : ti + 1],
                        xt16[:, 0:nb, ti, :],
                        start=(ti == 0),
                        stop=(ti == TI - 1),
                    )
                stage = wpool.tile([1, NB * D], F32, tag="stage")
                nc.scalar.activation(stage[0:1, 0 : nb * D], ps[0:1, 0 : nb * D], AF.Copy)
                nc.gpsimd.dma_start(
                    out=o_frac[tb][row0 : row0 + nb, :],
                    in_=stage[0:1, 0 : nb * D].rearrange("o (bi d) -> o bi d", bi=nb),
                )

            xh_r = xh.rearrange("(g bi) (p ti) d -> g p bi ti d", bi=NB, p=128)
            xh_r2 = xh.rearrange("(q bj) (p ti) d -> q p bj ti d", bj=2, p=128)
            for g in range(G):
                tb, gi = divmod(g, GT)
                if gi < GT - 1:
                    do_group(xh_r[g], NB, tb, NB * gi)
                    if gi == 23:
                        # rows 0..95 of this half are final once scatter g
                        # lands (engine partition slices must start 32-aligned);
                        # ship them while the remaining groups stream
                        o = wpool.tile([128, D], F32, tag="o")
                        nc.vector.tensor_add(
                            out=o[0:96], in0=o_base[tb][0:96], in1=o_frac[tb][0:96]
                        )
                        nc.gpsimd.dma_start(
                            out=out[tb * 128 : tb * 128 + 96, :], in_=o[0:96]
                        )
                else:
                    # last group split in two so the tail chain after the final
                    # DMA covers only 1 MiB of cast+matmul+scatter
                    for h in range(2):
                        do_group(xh_r2[64 * tb + 62 + h], 2, tb, 124 + 2 * h)
                    o2 = wpool.tile([128, D], F32, tag="o2")
                    nc.vector.tensor_add(
                        out=o2[96:128],
                        in0=o_base[tb][96:128],
                        in1=o_frac[tb][96:128],
                    )
                    nc.gpsimd.dma_start(
                        out=out[tb * 128 + 96 : tb * 128 + 128, :], in_=o2[96:128]
                    )

    nc.compile()
    return nc


_NC_CACHE = None


def _get_program() -> bass.Bass:
    global _NC_CACHE
    if _NC_CACHE is None:
        _NC_CACHE = _build_program()
    return _NC_CACHE


def _pack_consts(inputs: dict) -> np.ndarray:
    f = lambda x: np.asarray(x, dtype=np.float32)
    pk = np.zeros((128, _C_TOT), dtype=np.float32)
    pk[:, _C_DW1 : _C_DW1 + HID] = f(inputs["dw1"])
    pk[:, _C_DW2 : _C_DW2 + 2 * HID] = f(inputs["dw2"]).reshape(2, 128, HID).transpose(1, 0, 2).reshape(128, 2 * HID)
    pk[:, _C_DW3 : _C_DW3 + 2 * D] = f(inputs["dw3"]).reshape(2, 128, D).transpose(1, 0, 2).reshape(128, 2 * D)
    pk[:, _C_GW1 : _C_GW1 + HID] = f(inputs["gw1"])
    pk[:, _C_GW2 : _C_GW2 + 2 * HID] = f(inputs["gw2"]).reshape(2, 128, HID).transpose(1, 0, 2).reshape(128, 2 * HID)
    pk[:, _C_GW3 : _C_GW3 + 2 * D] = f(inputs["gw3"]).reshape(2, 128, D).transpose(1, 0, 2).reshape(128, 2 * D)
    bias = np.stack(
        [
            2.0 * f(inputs["db1"])[:128],
            2.0 * f(inputs["db1"])[128:],
            2.0 * f(inputs["db2"])[:128],
            2.0 * f(inputs["db2"])[128:],
            f(inputs["db3"]),
            2.0 * f(inputs["gb1"])[:128],
            2.0 * f(inputs["gb1"])[128:],
            2.0 * f(inputs["gb2"])[:128],
            2.0 * f(inputs["gb2"])[128:],
            f(inputs["gb3"]),
        ],
        axis=1,
    )
    pk[:, _C_BIAS : _C_BIAS + 10] = bias
    pk[:, _C_C8 : _C_C8 + TI] = _coeffs()
    pk[:, _C_IDENT : _C_IDENT + 128] = np.eye(128, dtype=np.float32)
    return pk


def _in_maps(inputs: dict) -> list[dict]:
    f = lambda x: np.ascontiguousarray(np.asarray(x, dtype=np.float32))
    xh = f(inputs["x_history"])
    xc = f(inputs["x_current"])
    nz = f(inputs["noise"])
    assert xh.shape == (B, K, D) and xc.shape == (B, D) and nz.shape == (B,)
    pk_base = _pack_consts(inputs)
    maps = []
    for c in range(N_CORES):
        s = slice(c * B_PER, (c + 1) * B_PER)
        pk = pk_base.copy()
        # xc tiles are stored [p, d] row-major per 128-row block
        pk[:, _C_XC : _C_XC + 128] = xc[s][:128]
        pk[:, _C_XC + 128 : _C_XC + 256] = xc[s][128:]
        pk[:, _C_NZ] = nz[s][:128]
        pk[:, _C_NZ + 1] = nz[s][128:]
        maps.append({"xh": xh[s], "pk": np.ascontiguousarray(pk)})
    return maps


def run(inputs: dict, trace: bool = False):
    nc = _get_program()
    res = run_bass_kernel_spmd(nc, _in_maps(inputs), list(range(N_CORES)), trace=trace)
    out = np.concatenate([res.results[c]["out"] for c in range(N_CORES)], axis=0)
    return out, res


def kernel(**inputs) -> np.ndarray:
    out, _ = run(inputs, trace=False)
    return out
